# revision 1
# baseline (speedup 1.0000x reference)
"""Multi-head self-attention (QK^T -> softmax -> ctx -> linear) on 8 TRN2 cores.

Sharding: each core owns one (batch, query-block) shard: batch = core//4,
queries [qlo, qlo+512) with qlo = (core%4)*512. Attention needs all keys of
the core's batch, so keys are replicated per batch; no collectives needed.

Math per core (head h, its 512 queries q, all 2048 keys k):
  S_T[k, q]   = sum_d x[k, h*64+d] * x[q, h*64+d]          (PE, f32r)
  P_T[k, q]   = exp(0.125 * S_T[k, q])                     (ACT, PSUM->SBUF)
  ctxT[m, q]  = sum_k xa[k, m] * P_T[k, q]                 (PE, accumulate)
     where xa = [x | ones] so row m=64 is the softmax denominator
  chunk[i, q] = ctxT[d, q] / ctxT[64, q]    (i = h*64+d)   (DVE)
  out[q, o]   = sum_i chunk[i, q]*W[o, i] + b[o]           (PE; bias via K=1 mm)

Everything stays in the transposed orientation so no on-chip transposes of the
attention matrix are ever needed, and ctxT directly feeds the final matmul as
the stationary operand.
"""

import sys

for _p in ("/opt/trn_rl_repo", "/root/.axon_site/_ro/trn_rl_repo"):
    if _p not in sys.path:
        sys.path.append(_p)

import numpy as np

import concourse.bacc as bacc
import concourse.bass as bass
import concourse.library_config as library_config
import concourse.mybir as mybir
import concourse.tile as tile

F32 = mybir.dt.float32
F32R = mybir.dt.float32r

B, L, H, NH, DH = 2, 2048, 1024, 16, 64
NCORES = 8


def build_nc(L=2048, NH=16, DH=64, QB=512, H=1024, use_f32r=True):
    """One SPMD Bass program; per-core data differences live in the inputs."""
    KBLKS = L // 128           # key blocks of 128
    NPAIR = NH // 2            # head pairs (2 heads = 128 partitions)
    AUG = DH + 1               # x augmented with a ones column
    HC = H // 128              # hidden chunks for the final matmul
    OBW = min(512, H)          # output column block width
    OBLKS = H // OBW
    QSUB = QB // 128
    SCALE = float(1.0 / np.sqrt(DH))
    mmdt = F32R if use_f32r else F32

    def r(ap):
        return ap if ap.dtype == mmdt else ap.bitcast(mmdt)

    nc = bacc.Bacc("TRN2")
    xt = nc.declare_dram_parameter("xt", [NPAIR, 128, L], mmdt, isOutput=False)
    xq = nc.declare_dram_parameter("xq", [NPAIR, 128, QB], mmdt, isOutput=False)
    xa = nc.declare_dram_parameter("xa", [NH, 128, KBLKS * AUG], mmdt, isOutput=False)
    wt = nc.declare_dram_parameter("wt", [128, HC * H], mmdt, isOutput=False)
    bias = nc.declare_dram_parameter("bias", [1, H], mmdt, isOutput=False)
    ones = nc.declare_dram_parameter("ones", [1, 128], mmdt, isOutput=False)
    out = nc.declare_dram_parameter("out", [QB, H], F32, isOutput=True)

    with tile.TileContext(nc) as tc:
        with (
            tc.tile_pool(name="xt", bufs=2) as xt_pool,
            tc.tile_pool(name="xq", bufs=2) as xq_pool,
            tc.tile_pool(name="xa", bufs=4) as xa_pool,
            tc.tile_pool(name="p", bufs=4) as p_pool,
            tc.tile_pool(name="consts", bufs=1) as consts,
            tc.tile_pool(name="ctxsb", bufs=NPAIR) as ctx_pool,
            tc.tile_pool(name="recip", bufs=4) as r_pool,
            tc.tile_pool(name="osb", bufs=3) as o_pool,
            tc.tile_pool(name="spsum", bufs=2, space="PSUM") as s_psum,
            tc.tile_pool(name="cpsum", bufs=2, space="PSUM") as c_psum,
        ):
            ones_t = consts.tile([1, 128], mmdt)
            nc.sync.dma_start(ones_t[:], ones[:])
            bias_t = consts.tile([1, H], mmdt)
            nc.sync.dma_start(bias_t[:], bias[:])

            # Pre-broadcast the bias across all 128 partitions once (PE ones
            # matmul) -> bias_bc holds b[o] in every row q.
            bias_bc = consts.tile([128, H], F32)
            for ob in range(OBLKS):
                obsl = slice(ob * OBW, (ob + 1) * OBW)
                bps = s_psum.tile([128, 2 * QB], F32, tag="s")
                nc.tensor.matmul(
                    bps[:, 0:OBW], r(ones_t[:, :]), r(bias_t[0:1, obsl]),
                    start=True, stop=True,
                )
                nc.vector.tensor_copy(bias_bc[:, obsl], bps[:, 0:OBW])

            # Output-projection weights are streamed one hidden-chunk per
            # head pair, so no big DMA ever stalls the attention pipeline.
            wt_ts = [consts.tile([128, H], mmdt, tag=f"wt{c}", name=f"wt{c}") for c in range(HC)]

            # Per-qs output accumulators: each chunk's projection contribution
            # is matmul'd into a briefly-held PSUM slot and DVE-added here,
            # one contribution per kb step of the following pair.
            acc = [consts.tile([128, H], F32, tag=f"acc{q}", name=f"acc{q}") for q in range(QSUB)]

            chunks = []
            contrib_q = []

            def emit_contrib(alt=None):
                c, qs, ob = contrib_q.pop(0)
                qsl = slice(qs * 128, (qs + 1) * 128)
                obsl = slice(ob * OBW, (ob + 1) * OBW)
                tag = ("ctx_a" if (qs + ob) % 2 == 0 else "ctx_b") if alt is None else alt
                cp = c_psum.tile([128, max(QB, OBW)], F32, tag=tag, name=f"cp{c}_{qs}_{ob}")
                nc.tensor.matmul(
                    cp[:, 0:OBW], r(chunks[c][:, qsl]), r(wt_ts[c][:, obsl]),
                    start=True, stop=True,
                )
                prevacc = bias_bc if c == 0 else acc[qs]
                nc.vector.tensor_add(
                    acc[qs][:, obsl], cp[:, 0:OBW], prevacc[:, obsl]
                )

            def emit_norm_p1(ctx_a, ctx_b):
                # Phase 1: reciprocals of the denominator rows + broadcast of
                # head A across DH partitions with a K=1 ones matmul.
                rc_a = r_pool.tile([1, QB], mmdt)
                rc_b = r_pool.tile([1, QB], mmdt)
                with nc.allow_low_precision(reason="f32r rounding for matmul"):
                    nc.vector.reciprocal(rc_a[:], ctx_a[DH : DH + 1, :])
                    nc.vector.reciprocal(rc_b[:], ctx_b[DH : DH + 1, :])
                bc_ps = s_psum.tile([128, 2 * QB], F32, tag="s")
                nc.tensor.matmul(
                    bc_ps[0:DH, 0:QB], r(ones_t[0:1, 0:DH]), rc_a[:],
                    start=True, stop=True,
                )
                return rc_b, bc_ps

            def emit_norm_p2(ctx_a, ctx_b, rc_b, bc_ps):
                # Phase 2 (next kb step, spreading PE load): broadcast head B,
                # then normalize ctxT into the SBUF chunk.
                nc.tensor.matmul(
                    bc_ps[0:DH, QB : 2 * QB], r(ones_t[0:1, 0:DH]), rc_b[:],
                    start=True, stop=True,
                )
                bc_sb = r_pool.tile([128, QB], F32, tag="bc")
                nc.vector.tensor_copy(bc_sb[0:DH, :], bc_ps[0:DH, 0:QB])
                nc.vector.tensor_copy(
                    bc_sb[DH : 2 * DH, :], bc_ps[0:DH, QB : 2 * QB]
                )
                chunk = ctx_pool.tile([128, QB], mmdt)
                nc.vector.tensor_mul(
                    chunk[0:DH, :], ctx_a[0:DH, :], bc_sb[0:DH, :]
                )
                nc.vector.tensor_mul(
                    chunk[DH : 2 * DH, :], ctx_b[0:DH, :],
                    bc_sb[DH : 2 * DH, :],
                )
                c = len(chunks)
                chunks.append(chunk)
                for qs in range(QSUB):
                    for ob in range(OBLKS):
                        contrib_q.append((c, qs, ob))

            def emit_norm(ctx_a, ctx_b):
                rc_b, bc_ps = emit_norm_p1(ctx_a, ctx_b)
                emit_norm_p2(ctx_a, ctx_b, rc_b, bc_ps)

            # One flat, globally software-pipelined stream over (pair, kb):
            # the ctx matmuls for global step t are emitted after the score
            # matmuls for step t+1, including across pair boundaries, so the
            # PE never waits on ACT's exp and ACT never waits on pair setup.
            NSTEP = NPAIR * KBLKS
            SKEW = 2 if KBLKS >= 4 else 1
            tiles = {}
            pipe = []
            for gs in range(NSTEP + SKEW):
                cur = None
                if gs < NSTEP:
                    pr, kb = divmod(gs, KBLKS)
                    if kb == 0:
                        xq_t = xq_pool.tile([128, QB], mmdt)
                        nc.sync.dma_start(xq_t[:], xq[pr])
                        xt_t = xt_pool.tile([128, L], mmdt)
                        # split the key DMA so the first score matmuls don't
                        # wait for the whole 1MB row block
                        nc.sync.dma_start(xt_t[:, 0 : L // 4], xt[pr][:, 0 : L // 4])
                        nc.sync.dma_start(xt_t[:, L // 4 :], xt[pr][:, L // 4 :])
                        xa_a = xa_pool.tile([128, KBLKS * AUG], mmdt)
                        nc.sync.dma_start(xa_a[:], xa[2 * pr])
                        xa_b = xa_pool.tile([128, KBLKS * AUG], mmdt)
                        nc.sync.dma_start(xa_b[:], xa[2 * pr + 1])
                        nc.sync.dma_start(
                            wt_ts[pr][:], wt[:, pr * H : (pr + 1) * H]
                        )
                        ctx_a = c_psum.tile([128, QB], F32)
                        ctx_b = c_psum.tile([128, QB], F32)
                        tiles[pr] = (xt_t, xq_t, xa_a, xa_b, ctx_a, ctx_b)
                    xt_t, xq_t, xa_a, xa_b, ctx_a, ctx_b = tiles[pr]
                    # norm phase 1 allocates its PSUM slot BEFORE this step's
                    # scores tile so the scores slot-rotation parity (which
                    # pipelines scores k+2 against exp k) is preserved.
                    if KBLKS >= 4 and pr > 0:
                        if kb == 2:
                            norm_state = emit_norm_p1(
                                tiles[pr - 1][4], tiles[pr - 1][5]
                            )
                        elif kb == 3:
                            emit_norm_p2(
                                tiles[pr - 1][4], tiles[pr - 1][5], *norm_state
                            )
                    elif kb == KBLKS - 1 and pr > 0:
                        emit_norm(tiles[pr - 1][4], tiles[pr - 1][5])
                    s_ab = s_psum.tile([128, 2 * QB], F32, tag="s")
                    ksl = slice(kb * 128, (kb + 1) * 128)
                    nc.tensor.matmul(
                        s_ab[:, 0:QB], r(xt_t[0:64, ksl]),
                        r(xq_t[0:64, :]), start=True, stop=True,
                    )
                    nc.tensor.matmul(
                        s_ab[:, QB : 2 * QB], r(xt_t[64:128, ksl]),
                        r(xq_t[64:128, :]), start=True, stop=True,
                    )
                    p_ab = p_pool.tile([128, 2 * QB], mmdt, tag="p")
                    nc.scalar.activation(
                        p_ab[:], s_ab[:], mybir.ActivationFunctionType.Exp,
                        scale=SCALE,
                    )
                    cur = (gs, p_ab)
                    if kb >= 5 and contrib_q:
                        emit_contrib()
                if cur is not None:
                    pipe.append(cur)
                prev = pipe.pop(0) if (len(pipe) > SKEW or cur is None) and pipe else None
                if prev is not None:
                    gsp, pp = prev
                    prp, kbp = divmod(gsp, KBLKS)
                    _, _, xa_a, xa_b, ctx_a, ctx_b = tiles[prp]
                    asl = slice(kbp * AUG, (kbp + 1) * AUG)
                    nc.tensor.matmul(
                        ctx_a[0:AUG, :], r(xa_a[:, asl]), r(pp[:, 0:QB]),
                        start=(kbp == 0), stop=(kbp == KBLKS - 1),
                    )
                    nc.tensor.matmul(
                        ctx_b[0:AUG, :], r(xa_b[:, asl]),
                        r(pp[:, QB : 2 * QB]),
                        start=(kbp == 0), stop=(kbp == KBLKS - 1),
                    )

            emit_norm(tiles[NPAIR - 1][4], tiles[NPAIR - 1][5])
            i = 0
            while contrib_q:
                emit_contrib(alt=["ctx_a", "ctx_b"][i % 2])
                i += 1
            for qs in range(QSUB):
                nc.sync.dma_start(out[qs * 128 : (qs + 1) * 128, :], acc[qs][:])
    nc.compile()
    return nc


def shard_inputs(key, W_ctx, b_ctx, L=2048, NH=16, DH=64, QB=512, H=1024):
    """Host-side prep of per-core input dicts."""
    KBLKS = L // 128
    NPAIR = NH // 2
    AUG = DH + 1
    HC = H // 128
    Bv = key.shape[0]
    ncores = NCORES
    qper = Bv * L // (ncores * QB)  # query blocks per batch... cores per batch
    cores_per_batch = ncores // Bv

    key = np.asarray(key, dtype=np.float32)
    xh = key.reshape(Bv, L, NH, DH)
    # xt: [B, NPAIR, 128, L], pair p rows 0:64 = head 2p, 64:128 = head 2p+1
    xt_full = np.ascontiguousarray(
        xh.transpose(0, 2, 3, 1).reshape(Bv, NPAIR, 2 * DH, L)
    )
    # xa: [B, NH, 128, KBLKS*AUG] with ones in column kb*AUG+DH
    xa_full = np.empty((Bv, NH, 128, KBLKS * AUG), dtype=np.float32)
    xa_view = xa_full.reshape(Bv, NH, 128, KBLKS, AUG)
    xa_view[..., DH] = 1.0
    # x natural per head, kb-blocked: [B, NH, KB, 128, DH] -> [B, NH, 128, KB, DH]
    xa_view[..., 0:DH] = xh.reshape(Bv, KBLKS, 128, NH, DH).transpose(
        0, 3, 2, 1, 4
    )
    wt_host = np.ascontiguousarray(
        np.asarray(W_ctx, np.float32).T.reshape(HC, 128, H).transpose(1, 0, 2)
        .reshape(128, HC * H)
    )
    bias_host = np.ascontiguousarray(np.asarray(b_ctx, np.float32).reshape(1, H))
    ones_host = np.ones((1, 128), dtype=np.float32)

    in_maps = []
    meta = []
    for c in range(ncores):
        b = c // cores_per_batch
        qlo = (c % cores_per_batch) * QB
        in_maps.append(
            {
                "xt": xt_full[b],
                "xq": np.ascontiguousarray(xt_full[b][:, :, qlo : qlo + QB]),
                "xa": xa_full[b],
                "wt": wt_host,
                "bias": bias_host,
                "ones": ones_host,
            }
        )
        meta.append((b, qlo))
    return in_maps, meta


_NC_CACHE = {}


def kernel(key, W_ctx, b_ctx):
    from concourse.bass_utils import run_bass_kernel_spmd

    key = np.asarray(key, dtype=np.float32)
    if "nc" not in _NC_CACHE:
        _NC_CACHE["nc"] = build_nc(L=L, NH=NH, DH=DH, QB=512, H=H)
    nc = _NC_CACHE["nc"]
    in_maps, meta = shard_inputs(key, W_ctx, b_ctx, L=L, NH=NH, DH=DH, QB=512, H=H)
    res = run_bass_kernel_spmd(nc, in_maps, list(range(NCORES)))
    outf = np.empty((B, L, H), dtype=np.float32)
    for c, (b, qlo) in enumerate(meta):
        outf[b, qlo : qlo + 512] = res.results[c]["out"]
    return outf



# revision 5
# speedup vs baseline: 1.0086x; 1.0086x over previous
"""Multi-head self-attention (QK^T -> softmax -> ctx -> linear) on 8 TRN2 cores.

Sharding: each core owns one (batch, query-block) shard: batch = core//4,
queries [qlo, qlo+512) with qlo = (core%4)*512. Attention needs all keys of
the core's batch, so keys are replicated per batch; no collectives needed.

Per core (head h, its 512 queries q, all 2048 keys k), all operands bf16:
  S_T[k, q]  = sum_d x[k, hd+d] * x[q, hd+d]            (PE, psum f32)
  P_T[k, q]  = exp(0.125 * S_T[k, q])                   (ACT, psum->sbuf bf16)
  ctx[q, m]  = sum_k P_T[k, q] * xa[k, m]               (PE; P parked as the
               stationary operand so only the 65-wide xa side streams;
               m = 64 dims + ones column -> softmax denominator at m=64)
  chunk[q,i] = ctx[q, d] / ctx[q, 64]                   (DVE tensor_scalar,
               per-partition recip scalar; -> bf16)
  chunkT     = transpose(chunk)                         (DMA xbar transpose)
  out[q, o]  = sum_i chunkT[i, q] * W[o, i] + b[o]      (PE + DVE adds)

The PE cost model charges only streamed output columns (stationary loads are
free), so parking P halves ctx cost vs streaming it; everything else is
orientation-chosen to keep output partitions full.
"""

import sys

for _p in ("/opt/trn_rl_repo", "/root/.axon_site/_ro/trn_rl_repo"):
    if _p not in sys.path:
        sys.path.append(_p)

import numpy as np

import concourse.bacc as bacc
import concourse.bass as bass
import concourse.mybir as mybir
import concourse.tile as tile

F32 = mybir.dt.float32
BF16 = mybir.dt.bfloat16

B, L, H, NH, DH = 2, 2048, 1024, 16, 64
NCORES = 8
QB = 512
KBLKS = L // 128          # 16 key blocks
NPAIR = NH // 2           # 8 head pairs
AUG = DH + 1              # 64 dims + ones column
HC = H // 128             # 8 hidden chunks (one per pair)
OBW = 512                 # proj output column block
SCALE = float(1.0 / np.sqrt(DH))
QSUB = QB // 128          # 4 query subtiles


def build_nc():
    nc = bacc.Bacc("TRN2")
    xt = nc.declare_dram_parameter("xt", [NPAIR, 128, L], BF16, isOutput=False)
    xq = nc.declare_dram_parameter("xq", [NPAIR, 128, QB], BF16, isOutput=False)
    xa = nc.declare_dram_parameter("xa", [NPAIR, 128, 2 * KBLKS * AUG], BF16, isOutput=False)
    wt = nc.declare_dram_parameter("wt", [128, HC * H], BF16, isOutput=False)
    bias = nc.declare_dram_parameter("bias", [1, H], BF16, isOutput=False)
    ones = nc.declare_dram_parameter("ones", [1, 128], BF16, isOutput=False)
    out = nc.declare_dram_parameter("out", [QB, H], F32, isOutput=True)

    NSTEP = NPAIR * KBLKS
    SKEW = 2

    with tile.TileContext(nc) as tc:
        with (
            tc.tile_pool(name="xt", bufs=2) as xt_pool,
            tc.tile_pool(name="xq", bufs=2) as xq_pool,
            tc.tile_pool(name="xa", bufs=2) as xa_pool,
            tc.tile_pool(name="p", bufs=4) as p_pool,
            tc.tile_pool(name="cq", bufs=8) as cq_pool,
            tc.tile_pool(name="rc", bufs=4) as rc_pool,
            tc.tile_pool(name="consts", bufs=1) as consts,
            tc.tile_pool(name="spsum", bufs=2, space="PSUM") as s_psum,
            tc.tile_pool(name="ctxpsum", bufs=3, space="PSUM") as ctx_psum,
            tc.tile_pool(name="prjpsum", bufs=1, space="PSUM") as prj_psum,
        ):
            ones_t = consts.tile([1, 128], BF16)
            nc.sync.dma_start(ones_t[:], ones[:])
            bias_t = consts.tile([1, H], BF16)
            nc.sync.dma_start(bias_t[:], bias[:])

            # bias broadcast across partitions via K=1 ones matmul
            bias_bc = consts.tile([128, H], F32)
            for ob in range(H // OBW):
                obsl = slice(ob * OBW, (ob + 1) * OBW)
                bps = s_psum.tile([128, 2 * QB], F32, tag="s")
                nc.tensor.matmul(
                    bps[:, 0:OBW], ones_t[0:1, :], bias_t[0:1, obsl],
                    start=True, stop=True,
                )
                nc.vector.tensor_copy(bias_bc[:, obsl], bps[:, 0:OBW])

            wt_ts = [
                consts.tile([128, H], BF16, tag=f"wt{c}", name=f"wt{c}")
                for c in range(HC)
            ]
            chunks = [
                consts.tile([128, QB], BF16, tag=f"ch{c}", name=f"ch{c}")
                for c in range(HC)
            ]
            acc = [
                consts.tile([128, H], F32, tag=f"acc{q}", name=f"acc{q}")
                for q in range(QSUB)
            ]

            def pair_dmas(pr):
                xt_t = xt_pool.tile([128, L], BF16, tag="xt", name=f"xt{pr}")
                nc.sync.dma_start(xt_t[:], xt[pr])
                xq_t = xq_pool.tile([128, QB], BF16, tag="xq", name=f"xq{pr}")
                nc.sync.dma_start(xq_t[:], xq[pr])
                xa_t = xa_pool.tile([128, 2 * KBLKS * AUG], BF16, tag="xa", name=f"xa{pr}")
                nc.sync.dma_start(xa_t[:], xa[pr])
                return [xt_t, xq_t, xa_t, None, None]

            def emit_proj_group(clist, qs, ob, pool, tag):
                obsl = slice(ob * OBW, (ob + 1) * OBW)
                qsl = slice(qs * 128, (qs + 1) * 128)
                cp = pool.tile([128, 2 * QB] if tag == "s" else [128, OBW],
                               F32, tag=tag, name=f"cp{clist[0]}_{qs}_{ob}")
                for i, c in enumerate(clist):
                    nc.tensor.matmul(
                        cp[:, 0:OBW], chunks[c][:, qsl], wt_ts[c][:, obsl],
                        start=(i == 0), stop=(i == len(clist) - 1),
                    )
                prev = bias_bc if clist[0] == 0 else acc[qs]
                nc.vector.tensor_add(acc[qs][:, obsl], cp[:, 0:OBW], prev[:, obsl])

            def emit_drain(prp, ctx_a, ctx_b):
                # normalize pair prp's ctx accumulators and transpose into
                # the proj chunk tile
                rc_a = rc_pool.tile([128, QSUB], F32, tag="rc", name=f"rca{prp}")
                rc_b = rc_pool.tile([128, QSUB], F32, tag="rc", name=f"rcb{prp}")
                with nc.allow_low_precision(reason="softmax denominator recip"):
                    nc.vector.reciprocal(rc_a[:], ctx_a[:, DH : QSUB * AUG : AUG])
                    nc.vector.reciprocal(rc_b[:], ctx_b[:, DH : QSUB * AUG : AUG])
                for qs in range(QSUB):
                    cq = cq_pool.tile([128, 128], BF16, tag="cq", name=f"cq{prp}_{qs}")
                    nc.vector.tensor_scalar_mul(
                        cq[:, 0:DH], ctx_a[:, qs * AUG : qs * AUG + DH],
                        rc_a[:, qs : qs + 1],
                    )
                    nc.vector.tensor_scalar_mul(
                        cq[:, DH : 2 * DH], ctx_b[:, qs * AUG : qs * AUG + DH],
                        rc_b[:, qs : qs + 1],
                    )
                    nc.sync.dma_start_transpose(
                        chunks[prp][:, qs * 128 : (qs + 1) * 128], cq[:, :]
                    )

            # proj emission plan: step -> (chunk list, qs, ob)
            # chunk-pairs (0,1)@pairs2-3, (2,3)@pairs4-5; chunk 4 singles@pair6,
            # (5,6)@pair7, chunk 7 + leftovers in the tail.
            plan = {}
            for hp, base_pr in ((0, 2), (1, 4)):
                cl = [2 * hp, 2 * hp + 1]
                gidx = 0
                for prw in (base_pr, base_pr + 1):
                    for kbw in (2, 6, 10, 14):
                        qs, ob = divmod(gidx, 2)
                        plan[prw * KBLKS + kbw] = (cl, qs, ob)
                        gidx += 1
            for i, kbw in enumerate((1, 3, 5, 7, 9, 11, 13, 15)):
                plan[6 * KBLKS + kbw] = ([4], i // 2, i % 2)
            pair56_tail = []
            for i, kbw in enumerate((4, 6, 8, 10, 12, 14)):
                plan[7 * KBLKS + kbw] = ([5, 6], i // 2, i % 2)
            pair56_tail = [([5, 6], 3, 0), ([5, 6], 3, 1)]

            tiles = {}
            pipe = []
            for gs in range(NSTEP + SKEW):
                cur = None
                if gs < NSTEP:
                    pr, kb = divmod(gs, KBLKS)
                    if pr == 0 and kb == 0:
                        tiles[0] = pair_dmas(0)
                        nc.sync.dma_start(wt_ts[0][:], wt[:, 0:H])
                    if kb == 2 and pr + 1 < NPAIR:
                        tiles[pr + 1] = pair_dmas(pr + 1)
                    if kb == 4 and pr + 1 < NPAIR:
                        nc.sync.dma_start(
                            wt_ts[pr + 1][:], wt[:, (pr + 1) * H : (pr + 2) * H]
                        )
                    xt_t, xq_t, xa_t, _, _ = tiles[pr]
                    ksl = slice(kb * 128, (kb + 1) * 128)
                    s_ab = s_psum.tile([128, 2 * QB], F32, tag="s")
                    nc.tensor.matmul(
                        s_ab[:, 0:QB], xt_t[0:64, ksl], xq_t[0:64, :],
                        start=True, stop=True,
                    )
                    nc.tensor.matmul(
                        s_ab[:, QB : 2 * QB], xt_t[64:128, ksl], xq_t[64:128, :],
                        start=True, stop=True,
                    )
                    p_ab = p_pool.tile([128, 2 * QB], BF16, tag="p")
                    nc.scalar.activation(
                        p_ab[:], s_ab[:], mybir.ActivationFunctionType.Exp,
                        scale=SCALE,
                    )
                    cur = (gs, p_ab)
                if cur is not None:
                    pipe.append(cur)
                prev = pipe.pop(0) if (len(pipe) > SKEW or cur is None) and pipe else None
                if prev is not None:
                    gsp, pp = prev
                    prp, kbp = divmod(gsp, KBLKS)
                    if kbp == 0:
                        ctx_a = ctx_psum.tile([128, QSUB * AUG], F32, tag="ctx",
                                              name=f"ctxa{prp}")
                        ctx_b = ctx_psum.tile([128, QSUB * AUG], F32, tag="ctx",
                                              name=f"ctxb{prp}")
                        tiles[prp][3] = ctx_a
                        tiles[prp][4] = ctx_b
                    _, _, xa_t, ctx_a, ctx_b = tiles[prp]
                    for j, ctx_t in ((0, ctx_a), (1, ctx_b)):
                        asl = slice(j * KBLKS * AUG + kbp * AUG,
                                    j * KBLKS * AUG + (kbp + 1) * AUG)
                        for qs in range(QSUB):
                            # The 4 qs accumulation groups share one psum
                            # bank (= one 2KB zero region): only the tile's
                            # first matmul may set start (start marks the
                            # WHOLE region pending-zero, so later groups'
                            # first writes still overwrite-not-accumulate),
                            # and only the last sets stop.
                            nc.tensor.matmul(
                                ctx_t[:, qs * AUG : (qs + 1) * AUG],
                                pp[:, j * QB + qs * 128 : j * QB + (qs + 1) * 128],
                                xa_t[:, asl],
                                start=(kbp == 0 and qs == 0),
                                stop=(kbp == KBLKS - 1 and qs == QSUB - 1),
                            )
                    if kbp == KBLKS - 1:
                        emit_drain(prp, ctx_a, ctx_b)
                    g = plan.get(gsp)
                    if g is not None:
                        emit_proj_group(g[0], g[1], g[2], prj_psum, "prj")

            # tail: leftover (5,6) groups, then chunk 7 qs-major with the out
            # DMA fired as soon as each qs's accumulator is final
            for cl, qs, ob in pair56_tail:
                emit_proj_group(cl, qs, ob, s_psum, "s")
            for qs in range(QSUB):
                for ob in range(2):
                    emit_proj_group([7], qs, ob, s_psum, "s")
                nc.sync.dma_start(out[qs * 128 : (qs + 1) * 128, :], acc[qs][:])
    nc.compile()
    return nc


def _to_bf16(x):
    import ml_dtypes

    return np.asarray(x, np.float32).astype(ml_dtypes.bfloat16)


def shard_inputs(key, W_ctx, b_ctx):
    """Host-side prep of per-core input dicts (bf16 layouts)."""
    Bv = key.shape[0]
    cores_per_batch = NCORES // Bv

    key = np.asarray(key, dtype=np.float32)
    xh = key.reshape(Bv, L, NH, DH)
    # xt: [B, NPAIR, 128, L]; pair p rows 0:64 = head 2p, 64:128 = head 2p+1
    xt_full = np.ascontiguousarray(
        xh.transpose(0, 2, 3, 1).reshape(Bv, NPAIR, 2 * DH, L)
    )
    # xa: [B, NH, 128, KBLKS*AUG] with ones in column kb*AUG+DH, then pair-merged
    xa_full = np.empty((Bv, NH, 128, KBLKS * AUG), dtype=np.float32)
    xa_view = xa_full.reshape(Bv, NH, 128, KBLKS, AUG)
    xa_view[..., DH] = 1.0
    xa_view[..., 0:DH] = xh.reshape(Bv, KBLKS, 128, NH, DH).transpose(0, 3, 2, 1, 4)
    # [B, NPAIR, 2, 128, KA] -> [B, NPAIR, 128, 2*KA]
    ka = KBLKS * AUG
    xa_pair = np.ascontiguousarray(
        xa_full.reshape(Bv, NPAIR, 2, 128, ka).transpose(0, 1, 3, 2, 4)
        .reshape(Bv, NPAIR, 128, 2 * ka)
    )
    wt_host = np.ascontiguousarray(
        np.asarray(W_ctx, np.float32).T.reshape(HC, 128, H).transpose(1, 0, 2)
        .reshape(128, HC * H)
    )
    bias_host = np.asarray(b_ctx, np.float32).reshape(1, H)
    ones_host = np.ones((1, 128), dtype=np.float32)

    xt_b = _to_bf16(xt_full)
    xa_b = _to_bf16(xa_pair)
    wt_b = _to_bf16(wt_host)
    bias_b = _to_bf16(bias_host)
    ones_b = _to_bf16(ones_host)

    in_maps = []
    meta = []
    for c in range(NCORES):
        b = c // cores_per_batch
        qlo = (c % cores_per_batch) * QB
        in_maps.append(
            {
                "xt": xt_b[b],
                "xq": np.ascontiguousarray(xt_b[b][:, :, qlo : qlo + QB]),
                "xa": xa_b[b],
                "wt": wt_b,
                "bias": bias_b,
                "ones": ones_b,
            }
        )
        meta.append((b, qlo))
    return in_maps, meta


_NC_CACHE = {}


def kernel(key, W_ctx, b_ctx):
    from concourse.bass_utils import run_bass_kernel_spmd

    key = np.asarray(key, dtype=np.float32)
    if "nc" not in _NC_CACHE:
        _NC_CACHE["nc"] = build_nc()
    nc = _NC_CACHE["nc"]
    in_maps, meta = shard_inputs(key, W_ctx, b_ctx)
    res = run_bass_kernel_spmd(nc, in_maps, list(range(NCORES)))
    outf = np.empty((B, L, H), dtype=np.float32)
    for c, (b, qlo) in enumerate(meta):
        outf[b, qlo : qlo + QB] = res.results[c]["out"]
    return outf


# revision 13
# speedup vs baseline: 1.0391x; 1.0302x over previous
"""Multi-head self-attention (QK^T -> softmax -> ctx -> linear) on 8 TRN2 cores.

Sharding: each core owns one (batch, query-block) shard: batch = core//4,
queries [qlo, qlo+512) with qlo = (core%4)*512. Attention needs all keys of
the core's batch, so keys are replicated per batch; no collectives needed.

Per core (head h, its 512 queries q, all 2048 keys k), all operands bf16:
  S_T[k, q]  = sum_d x[k, hd+d] * x[q, hd+d]            (PE, psum f32)
  P_T[k, q]  = exp(0.125 * S_T[k, q])                   (ACT, psum->sbuf bf16)
  ctx[q, m]  = sum_k P_T[k, q] * xa[k, m]               (PE; P parked as the
               stationary operand so only the 65-wide xa side streams;
               m = 64 dims + ones column -> softmax denominator at m=64)
  chunk[q,i] = ctx[q, d] / ctx[q, 64]                   (DVE tensor_scalar,
               per-partition recip scalar; -> bf16)
  chunkT     = transpose(chunk)                         (DMA xbar transpose)
  out[q, o]  = sum_i chunkT[i, q] * W[o, i] + b[o]      (PE + DVE adds)

The PE cost model charges only streamed output columns (stationary loads are
free), so parking P halves ctx cost vs streaming it; everything else is
orientation-chosen to keep output partitions full.
"""

import sys

for _p in ("/opt/trn_rl_repo", "/root/.axon_site/_ro/trn_rl_repo"):
    if _p not in sys.path:
        sys.path.append(_p)

import numpy as np

import concourse.bacc as bacc
import concourse.bass as bass
import concourse.mybir as mybir
import concourse.tile as tile

F32 = mybir.dt.float32
BF16 = mybir.dt.bfloat16

B, L, H, NH, DH = 2, 2048, 1024, 16, 64
NCORES = 8
QB = 512
KBLKS = L // 128          # 16 key blocks
NPAIR = NH // 2           # 8 head pairs
AUG = DH + 1              # 64 dims + ones column
HC = H // 128             # 8 hidden chunks (one per pair)
OBW = 512                 # proj output column block
SCALE = float(1.0 / np.sqrt(DH))
QSUB = QB // 128          # 4 query subtiles


def build_nc():
    nc = bacc.Bacc("TRN2")
    xt = nc.declare_dram_parameter("xt", [NPAIR, 128, L], BF16, isOutput=False)
    xq = nc.declare_dram_parameter("xq", [NPAIR, 128, QB], BF16, isOutput=False)
    xa = nc.declare_dram_parameter("xa", [NPAIR, 128, 2 * KBLKS * AUG], BF16, isOutput=False)
    wt = nc.declare_dram_parameter("wt", [128, HC * H], BF16, isOutput=False)
    bias = nc.declare_dram_parameter("bias", [1, H], BF16, isOutput=False)
    ones = nc.declare_dram_parameter("ones", [1, 128], BF16, isOutput=False)
    out = nc.declare_dram_parameter("out", [QB, H], F32, isOutput=True)

    NSTEP = NPAIR * KBLKS
    SKEW = 2
    # scheduling floor per global step (ns): keeps the tile scheduler from
    # hoisting proj work into much earlier PE positions, where an unmet
    # transpose dep would stall the PE counter that gates the exp stream.
    TSTEP_NS = 1000.0
    TBASE_NS = 3000.0

    def floor_ms(gs):
        return (TBASE_NS + gs * TSTEP_NS) / 1e6

    with tile.TileContext(nc) as tc:
        with (
            tc.tile_pool(name="xt", bufs=2) as xt_pool,
            tc.tile_pool(name="xq", bufs=2) as xq_pool,
            tc.tile_pool(name="xa", bufs=2) as xa_pool,
            tc.tile_pool(name="p", bufs=4) as p_pool,
            tc.tile_pool(name="cq", bufs=8) as cq_pool,
            tc.tile_pool(name="rc", bufs=4) as rc_pool,
            tc.tile_pool(name="consts", bufs=1) as consts,
            tc.tile_pool(name="spsum", bufs=2, space="PSUM") as s_psum,
            tc.tile_pool(name="ctxpsum", bufs=3, space="PSUM") as ctx_psum,
            tc.tile_pool(name="prjpsum", bufs=1, space="PSUM") as prj_psum,
        ):
            ones_t = consts.tile([1, 128], BF16)
            bias_t = consts.tile([1, H], BF16)
            bias_bc = consts.tile([128, H], F32)

            wt_ts = [
                consts.tile([128, H], BF16, tag=f"wt{c}", name=f"wt{c}")
                for c in range(HC)
            ]
            chunks = [
                consts.tile([128, QB], BF16, tag=f"ch{c}", name=f"ch{c}")
                for c in range(HC)
            ]
            acc = [
                consts.tile([128, H], F32, tag=f"acc{q}", name=f"acc{q}")
                for q in range(QSUB)
            ]

            def pair_dmas(pr, split_xt=False):
                xt_t = xt_pool.tile([128, L], BF16, tag="xt", name=f"xt{pr}")
                xq_t = xq_pool.tile([128, QB], BF16, tag="xq", name=f"xq{pr}")
                nc.sync.dma_start(xq_t[:], xq[pr])
                if split_xt:
                    # first key quarter lands fast so scores can start early
                    nc.sync.dma_start(xt_t[:, 0:512], xt[pr][:, 0:512])
                    nc.sync.dma_start(xt_t[:, 512:L], xt[pr][:, 512:L])
                else:
                    nc.sync.dma_start(xt_t[:], xt[pr])
                xa_t = xa_pool.tile([128, 2 * KBLKS * AUG], BF16, tag="xa", name=f"xa{pr}")
                nc.sync.dma_start(xa_t[:], xa[pr])
                return [xt_t, xq_t, xa_t, None, None]

            def emit_proj_group(clist, qs, ob, pool, tag, eng=None):
                obsl = slice(ob * OBW, (ob + 1) * OBW)
                qsl = slice(qs * 128, (qs + 1) * 128)
                cp = pool.tile([128, 2 * QB] if tag == "s" else [128, OBW],
                               F32, tag=tag, name=f"cp{clist[0]}_{qs}_{ob}")
                for i, c in enumerate(clist):
                    nc.tensor.matmul(
                        cp[:, 0:OBW], chunks[c][:, qsl], wt_ts[c][:, obsl],
                        start=(i == 0), stop=(i == len(clist) - 1),
                    )
                prev = bias_bc if clist[0] == 0 else acc[qs]
                (eng or nc.vector).tensor_add(
                    acc[qs][:, obsl], cp[:, 0:OBW], prev[:, obsl]
                )

            def emit_drain(prp, ctx_a, ctx_b):
                # normalize pair prp's ctx accumulators and transpose into
                # the proj chunk tile; for the final pair, pipeline the
                # remaining projection groups and output DMAs per q-subtile
                last = prp == NPAIR - 1
                rc_a = rc_pool.tile([128, QSUB], F32, tag="rc", name=f"rca{prp}")
                rc_b = rc_pool.tile([128, QSUB], F32, tag="rc", name=f"rcb{prp}")
                with nc.allow_low_precision(reason="softmax denominator recip"):
                    nc.vector.reciprocal(rc_a[:], ctx_a[:, DH : QSUB * AUG : AUG])
                    nc.vector.reciprocal(rc_b[:], ctx_b[:, DH : QSUB * AUG : AUG])
                for qs in range(QSUB):
                    cq = cq_pool.tile([128, 128], BF16, tag="cq", name=f"cq{prp}_{qs}")
                    nc.vector.tensor_scalar_mul(
                        cq[:, 0:DH], ctx_a[:, qs * AUG : qs * AUG + DH],
                        rc_a[:, qs : qs + 1],
                    )
                    nc.vector.tensor_scalar_mul(
                        cq[:, DH : 2 * DH], ctx_b[:, qs * AUG : qs * AUG + DH],
                        rc_b[:, qs : qs + 1],
                    )
                    nc.sync.dma_start_transpose(
                        chunks[prp][:, qs * 128 : (qs + 1) * 128], cq[:, :]
                    )
                    if last:
                        for cl, ob, eng in tail_plan[qs]:
                            emit_proj_group(cl, qs, ob, s_psum, "s", eng=eng)
                            if cl[0] == HC - 1:
                                obsl = slice(ob * OBW, (ob + 1) * OBW)
                                nc.sync.dma_start(
                                    out[qs * 128 : (qs + 1) * 128, obsl],
                                    acc[qs][:, obsl],
                                )

            # proj emission plan: step -> (chunk list, qs, ob)
            # chunk-pairs (0,1)@pairs2-3, (2,3)@pairs4-5; chunk 4 singles@pair6,
            # (5,6)@pair7, chunk 7 + leftovers in the tail.
            plan = {}
            for hp, base_pr in ((0, 2), (1, 4)):
                cl = [2 * hp, 2 * hp + 1]
                gidx = 0
                for prw in (base_pr, base_pr + 1):
                    for kbw in (2, 6, 10, 14):
                        qs, ob = divmod(gidx, 2)
                        plan[prw * KBLKS + kbw] = (cl, qs, ob)
                        gidx += 1
            for i, kbw in enumerate((1, 3, 5, 7, 9, 11, 13, 15)):
                plan[6 * KBLKS + kbw] = ([4], i // 2, i % 2)
            for i, kbw in enumerate((4, 6, 8, 10, 12, 14)):
                plan[7 * KBLKS + kbw] = ([5, 6], i // 2, i % 2)
            # per-qs tail work, emitted interleaved with the final drain:
            # alternate add engines so the DVE chain isn't serial
            tail_plan = {
                0: [([7], 0, None), ([7], 1, None)],
                1: [([7], 0, None), ([7], 1, None)],
                2: [([7], 0, None), ([7], 1, None)],
                3: [([5, 6], 0, None), ([5, 6], 1, None),
                    ([7], 0, None), ([7], 1, None)],
            }

            tiles = {}
            pipe = []
            for gs in range(NSTEP + SKEW):
                cur = None
                if gs < NSTEP:
                    pr, kb = divmod(gs, KBLKS)
                    if pr == 0 and kb == 0:
                        tiles[0] = pair_dmas(0, split_xt=True)
                        nc.sync.dma_start(wt_ts[0][:], wt[:, 0:H])
                        nc.sync.dma_start(ones_t[:], ones[:])
                        nc.sync.dma_start(bias_t[:], bias[:])
                    if pr == 0 and kb == 1:
                        # bias broadcast across partitions via K=1 ones matmul
                        # (emitted after the first score step; its DMAs land
                        # behind the pair-0 loads on the HWDGE queue)
                        for ob in range(H // OBW):
                            obsl = slice(ob * OBW, (ob + 1) * OBW)
                            bps = s_psum.tile([128, 2 * QB], F32, tag="s")
                            nc.tensor.matmul(
                                bps[:, 0:OBW], ones_t[0:1, :], bias_t[0:1, obsl],
                                start=True, stop=True,
                            )
                            nc.vector.tensor_copy(bias_bc[:, obsl], bps[:, 0:OBW])
                    if kb == 2 and pr + 1 < NPAIR:
                        tiles[pr + 1] = pair_dmas(pr + 1)
                    if kb == 4 and pr + 1 < NPAIR:
                        nc.sync.dma_start(
                            wt_ts[pr + 1][:], wt[:, (pr + 1) * H : (pr + 2) * H]
                        )
                    xt_t, xq_t, xa_t, _, _ = tiles[pr]
                    ksl = slice(kb * 128, (kb + 1) * 128)
                    s_ab = s_psum.tile([128, 2 * QB], F32, tag="s")
                    nc.tensor.matmul(
                        s_ab[:, 0:QB], xt_t[0:64, ksl], xq_t[0:64, :],
                        start=True, stop=True,
                    )
                    nc.tensor.matmul(
                        s_ab[:, QB : 2 * QB], xt_t[64:128, ksl], xq_t[64:128, :],
                        start=True, stop=True,
                    )
                    p_ab = p_pool.tile([128, 2 * QB], BF16, tag="p")
                    nc.scalar.activation(
                        p_ab[:], s_ab[:], mybir.ActivationFunctionType.Exp,
                        scale=SCALE,
                    )
                    cur = (gs, p_ab)
                if cur is not None:
                    pipe.append(cur)
                prev = pipe.pop(0) if (len(pipe) > SKEW or cur is None) and pipe else None
                if prev is not None:
                    gsp, pp = prev
                    prp, kbp = divmod(gsp, KBLKS)
                    if kbp == 0:
                        ctx_a = ctx_psum.tile([128, QSUB * AUG], F32, tag="ctx",
                                              name=f"ctxa{prp}")
                        ctx_b = ctx_psum.tile([128, QSUB * AUG], F32, tag="ctx",
                                              name=f"ctxb{prp}")
                        tiles[prp][3] = ctx_a
                        tiles[prp][4] = ctx_b
                    _, _, xa_t, ctx_a, ctx_b = tiles[prp]
                    for j, ctx_t in ((0, ctx_a), (1, ctx_b)):
                        asl = slice(j * KBLKS * AUG + kbp * AUG,
                                    j * KBLKS * AUG + (kbp + 1) * AUG)
                        for qs in range(QSUB):
                            # The 4 qs accumulation groups share one psum
                            # bank (= one 2KB zero region): only the tile's
                            # first matmul may set start (start marks the
                            # WHOLE region pending-zero, so later groups'
                            # first writes still overwrite-not-accumulate),
                            # and only the last sets stop.
                            nc.tensor.matmul(
                                ctx_t[:, qs * AUG : (qs + 1) * AUG],
                                pp[:, j * QB + qs * 128 : j * QB + (qs + 1) * 128],
                                xa_t[:, asl],
                                start=(kbp == 0 and qs == 0),
                                stop=(kbp == KBLKS - 1 and qs == QSUB - 1),
                            )
                    if kbp == KBLKS - 1:
                        emit_drain(prp, ctx_a, ctx_b)
                    g = plan.get(gsp)
                    if g is not None:
                        with tc.tile_wait_until(floor_ms(gsp)):
                            emit_proj_group(g[0], g[1], g[2], prj_psum, "prj")
    nc.compile()
    return nc


def _to_bf16(x):
    import ml_dtypes

    return np.asarray(x, np.float32).astype(ml_dtypes.bfloat16)


def shard_inputs(key, W_ctx, b_ctx):
    """Host-side prep of per-core input dicts (bf16 layouts)."""
    Bv = key.shape[0]
    cores_per_batch = NCORES // Bv

    key = np.asarray(key, dtype=np.float32)
    xh = key.reshape(Bv, L, NH, DH)
    # xt: [B, NPAIR, 128, L]; pair p rows 0:64 = head 2p, 64:128 = head 2p+1
    xt_full = np.ascontiguousarray(
        xh.transpose(0, 2, 3, 1).reshape(Bv, NPAIR, 2 * DH, L)
    )
    # xa: [B, NH, 128, KBLKS*AUG] with ones in column kb*AUG+DH, then pair-merged
    xa_full = np.empty((Bv, NH, 128, KBLKS * AUG), dtype=np.float32)
    xa_view = xa_full.reshape(Bv, NH, 128, KBLKS, AUG)
    xa_view[..., DH] = 1.0
    xa_view[..., 0:DH] = xh.reshape(Bv, KBLKS, 128, NH, DH).transpose(0, 3, 2, 1, 4)
    # [B, NPAIR, 2, 128, KA] -> [B, NPAIR, 128, 2*KA]
    ka = KBLKS * AUG
    xa_pair = np.ascontiguousarray(
        xa_full.reshape(Bv, NPAIR, 2, 128, ka).transpose(0, 1, 3, 2, 4)
        .reshape(Bv, NPAIR, 128, 2 * ka)
    )
    wt_host = np.ascontiguousarray(
        np.asarray(W_ctx, np.float32).T.reshape(HC, 128, H).transpose(1, 0, 2)
        .reshape(128, HC * H)
    )
    bias_host = np.asarray(b_ctx, np.float32).reshape(1, H)
    ones_host = np.ones((1, 128), dtype=np.float32)

    xt_b = _to_bf16(xt_full)
    xa_b = _to_bf16(xa_pair)
    wt_b = _to_bf16(wt_host)
    bias_b = _to_bf16(bias_host)
    ones_b = _to_bf16(ones_host)

    in_maps = []
    meta = []
    for c in range(NCORES):
        b = c // cores_per_batch
        qlo = (c % cores_per_batch) * QB
        in_maps.append(
            {
                "xt": xt_b[b],
                "xq": np.ascontiguousarray(xt_b[b][:, :, qlo : qlo + QB]),
                "xa": xa_b[b],
                "wt": wt_b,
                "bias": bias_b,
                "ones": ones_b,
            }
        )
        meta.append((b, qlo))
    return in_maps, meta


_NC_CACHE = {}


def kernel(key, W_ctx, b_ctx):
    from concourse.bass_utils import run_bass_kernel_spmd

    key = np.asarray(key, dtype=np.float32)
    if "nc" not in _NC_CACHE:
        _NC_CACHE["nc"] = build_nc()
    nc = _NC_CACHE["nc"]
    in_maps, meta = shard_inputs(key, W_ctx, b_ctx)
    res = run_bass_kernel_spmd(nc, in_maps, list(range(NCORES)))
    outf = np.empty((B, L, H), dtype=np.float32)
    for c, (b, qlo) in enumerate(meta):
        outf[b, qlo : qlo + QB] = res.results[c]["out"]
    return outf


# revision 16
# speedup vs baseline: 1.0589x; 1.0191x over previous
"""Multi-head self-attention (QK^T -> softmax -> ctx -> linear) on 8 TRN2 cores.

Sharding: each core owns one (batch, query-block) shard: batch = core//4,
queries [qlo, qlo+512) with qlo = (core%4)*512. Attention needs all keys of
the core's batch, so keys are replicated per batch; no collectives needed.

Per core (head h, its 512 queries q, all 2048 keys k), all operands bf16:
  S_T[k, q]  = sum_d x[k, hd+d] * x[q, hd+d]            (PE, psum f32)
  P_T[k, q]  = exp(0.125 * S_T[k, q])                   (ACT, psum->sbuf bf16)
  ctx[q, m]  = sum_k P_T[k, q] * xa[k, m]               (PE; P parked as the
               stationary operand so only the 65-wide xa side streams;
               m = 64 dims + ones column -> softmax denominator at m=64)
  chunk[q,i] = ctx[q, d] / ctx[q, 64]                   (DVE tensor_scalar,
               per-partition recip scalar; -> bf16)
  chunkT     = transpose(chunk)                         (DMA xbar transpose)
  out[q, o]  = sum_i chunkT[i, q] * W[o, i] + b[o]      (PE + DVE adds)

The PE cost model charges only streamed output columns (stationary loads are
free), so parking P halves ctx cost vs streaming it; everything else is
orientation-chosen to keep output partitions full.
"""

import sys

for _p in ("/opt/trn_rl_repo", "/root/.axon_site/_ro/trn_rl_repo"):
    if _p not in sys.path:
        sys.path.append(_p)

import numpy as np

import concourse.bacc as bacc
import concourse.bass as bass
import concourse.mybir as mybir
import concourse.tile as tile

F32 = mybir.dt.float32
BF16 = mybir.dt.bfloat16

B, L, H, NH, DH = 2, 2048, 1024, 16, 64
NCORES = 8
QB = 512
KBLKS = L // 128          # 16 key blocks
NPAIR = NH // 2           # 8 head pairs
AUG = DH + 1              # 64 dims + ones column
HC = H // 128             # 8 hidden chunks (one per pair)
OBW = 512                 # proj output column block
SCALE = float(1.0 / np.sqrt(DH))
QSUB = QB // 128          # 4 query subtiles


def build_nc():
    nc = bacc.Bacc("TRN2")
    xt = nc.declare_dram_parameter("xt", [NPAIR, 128, L], BF16, isOutput=False)
    xq = nc.declare_dram_parameter("xq", [NPAIR, 128, QB], BF16, isOutput=False)
    xa = nc.declare_dram_parameter("xa", [NPAIR, 128, 2 * KBLKS * AUG], BF16, isOutput=False)
    wt = nc.declare_dram_parameter("wt", [128, HC * H], BF16, isOutput=False)
    bias = nc.declare_dram_parameter("bias", [1, H], BF16, isOutput=False)
    ones = nc.declare_dram_parameter("ones", [1, 128], BF16, isOutput=False)
    out = nc.declare_dram_parameter("out", [QB, H], F32, isOutput=True)

    NSTEP = NPAIR * KBLKS
    SKEW = 2
    # scheduling floor per global step (ns): keeps the tile scheduler from
    # hoisting proj work into much earlier PE positions, where an unmet
    # transpose dep would stall the PE counter that gates the exp stream.
    TSTEP_NS = 1000.0
    TBASE_NS = 3000.0

    def floor_ms(gs):
        return (TBASE_NS + gs * TSTEP_NS) / 1e6

    with tile.TileContext(nc) as tc:
        with (
            tc.tile_pool(name="xt", bufs=2) as xt_pool,
            tc.tile_pool(name="xq", bufs=2) as xq_pool,
            tc.tile_pool(name="xa", bufs=2) as xa_pool,
            tc.tile_pool(name="p", bufs=4) as p_pool,
            tc.tile_pool(name="cq", bufs=8) as cq_pool,
            tc.tile_pool(name="rc", bufs=4) as rc_pool,
            tc.tile_pool(name="consts", bufs=1) as consts,
            tc.tile_pool(name="spsum", bufs=2, space="PSUM") as s_psum,
            tc.tile_pool(name="ctxpsum", bufs=3, space="PSUM") as ctx_psum,
            tc.tile_pool(name="prjpsum", bufs=1, space="PSUM") as prj_psum,
        ):
            ones_t = consts.tile([1, 128], BF16)
            bias_t = consts.tile([1, H], BF16)
            bias_bc = consts.tile([128, H], F32)

            wt_ts = [
                consts.tile([128, H], BF16, tag=f"wt{c}", name=f"wt{c}")
                for c in range(HC)
            ]
            chunks = [
                consts.tile([128, QB], BF16, tag=f"ch{c}", name=f"ch{c}")
                for c in range(HC)
            ]
            acc = [
                consts.tile([128, H], F32, tag=f"acc{q}", name=f"acc{q}")
                for q in range(QSUB)
            ]

            def pair_dmas(pr, split_xt=False):
                xt_t = xt_pool.tile([128, L], BF16, tag="xt", name=f"xt{pr}")
                xq_t = xq_pool.tile([128, QB], BF16, tag="xq", name=f"xq{pr}")
                nc.sync.dma_start(xq_t[:], xq[pr])
                if split_xt:
                    # first key quarter lands fast so scores can start early
                    nc.sync.dma_start(xt_t[:, 0:512], xt[pr][:, 0:512])
                    nc.sync.dma_start(xt_t[:, 512:L], xt[pr][:, 512:L])
                else:
                    nc.sync.dma_start(xt_t[:], xt[pr])
                xa_t = xa_pool.tile([128, 2 * KBLKS * AUG], BF16, tag="xa", name=f"xa{pr}")
                nc.sync.dma_start(xa_t[:], xa[pr])
                return [xt_t, xq_t, xa_t, None, None]

            def emit_proj_group(clist, qs, ob, pool, tag, eng=None):
                obsl = slice(ob * OBW, (ob + 1) * OBW)
                qsl = slice(qs * 128, (qs + 1) * 128)
                cp = pool.tile([128, 2 * QB] if tag == "s" else [128, OBW],
                               F32, tag=tag, name=f"cp{clist[0]}_{qs}_{ob}")
                for i, c in enumerate(clist):
                    nc.tensor.matmul(
                        cp[:, 0:OBW], chunks[c][:, qsl], wt_ts[c][:, obsl],
                        start=(i == 0), stop=(i == len(clist) - 1),
                    )
                prev = bias_bc if clist[0] == 0 else acc[qs]
                (eng or nc.vector).tensor_add(
                    acc[qs][:, obsl], cp[:, 0:OBW], prev[:, obsl]
                )

            def emit_drain(prp, ctx_a, ctx_b):
                # normalize pair prp's ctx accumulators and transpose into
                # the proj chunk tile; for the final pair, pipeline the
                # remaining projection groups and output DMAs per q-subtile
                last = prp == NPAIR - 1
                rc_a = rc_pool.tile([128, QSUB], F32, tag="rc", name=f"rca{prp}")
                rc_b = rc_pool.tile([128, QSUB], F32, tag="rc", name=f"rcb{prp}")
                with nc.allow_low_precision(reason="softmax denominator recip"):
                    nc.vector.reciprocal(rc_a[:], ctx_a[:, DH : QSUB * AUG : AUG])
                    nc.vector.reciprocal(rc_b[:], ctx_b[:, DH : QSUB * AUG : AUG])
                for qs in range(QSUB):
                    cq = cq_pool.tile([128, 128], BF16, tag="cq", name=f"cq{prp}_{qs}")
                    nc.vector.tensor_scalar_mul(
                        cq[:, 0:DH], ctx_a[:, qs * AUG : qs * AUG + DH],
                        rc_a[:, qs : qs + 1],
                    )
                    nc.vector.tensor_scalar_mul(
                        cq[:, DH : 2 * DH], ctx_b[:, qs * AUG : qs * AUG + DH],
                        rc_b[:, qs : qs + 1],
                    )
                    nc.sync.dma_start_transpose(
                        chunks[prp][:, qs * 128 : (qs + 1) * 128], cq[:, :]
                    )
                    if last:
                        for cl, ob, eng in tail_plan[qs]:
                            emit_proj_group(cl, qs, ob, s_psum, "s", eng=eng)
                            if cl[0] == HC - 1:
                                obsl = slice(ob * OBW, (ob + 1) * OBW)
                                nc.sync.dma_start(
                                    out[qs * 128 : (qs + 1) * 128, obsl],
                                    acc[qs][:, obsl],
                                )

            # proj emission plan: step -> (chunk list, qs, ob)
            # chunk-pairs (0,1)@pairs2-3, (2,3)@pairs4-5; chunk 4 singles@pair6,
            # (5,6)@pair7, chunk 7 + leftovers in the tail.
            # window steps start ~4 kbs after the newer chunk's drain so the
            # first group never waits on an in-flight transpose
            plan = {}
            for hp, base_pr in ((0, 2), (1, 4)):
                cl = [2 * hp, 2 * hp + 1]
                steps = [base_pr * KBLKS + k for k in (6, 8, 10, 12, 14)] + [
                    (base_pr + 1) * KBLKS + k for k in (4, 8, 12)
                ]
                for gidx, st in enumerate(steps):
                    qs, ob = divmod(gidx, 2)
                    plan[st] = (cl, qs, ob)
            for i, kbw in enumerate((3, 5, 7, 9, 11, 13, 15)):
                plan[6 * KBLKS + kbw] = ([4], i // 2, i % 2)
            plan[7 * KBLKS + 1] = ([4], 3, 1)
            for i, kbw in enumerate((5, 7, 9, 11, 13, 15)):
                plan[7 * KBLKS + kbw] = ([5, 6], i // 2, i % 2)
            # per-qs tail work, emitted interleaved with the final drain:
            # alternate add engines so the DVE chain isn't serial
            tail_plan = {
                0: [([7], 0, None), ([7], 1, None)],
                1: [([7], 0, None), ([7], 1, None)],
                2: [([7], 0, None), ([7], 1, None)],
                3: [([5, 6], 0, None), ([5, 6], 1, None),
                    ([7], 0, None), ([7], 1, None)],
            }

            tiles = {}
            pipe = []
            for gs in range(NSTEP + SKEW):
                cur = None
                if gs < NSTEP:
                    pr, kb = divmod(gs, KBLKS)
                    if pr == 0 and kb == 0:
                        tiles[0] = pair_dmas(0, split_xt=True)
                        nc.sync.dma_start(wt_ts[0][:], wt[:, 0:H])
                        nc.sync.dma_start(ones_t[:], ones[:])
                        nc.sync.dma_start(bias_t[:], bias[:])
                    if pr == 0 and kb == 1:
                        # bias broadcast across partitions via K=1 ones matmul
                        # (uses the proj psum bank, idle until pair 2, so the
                        # late-landing bias DMA never stalls the score slots)
                        for ob in range(H // OBW):
                            obsl = slice(ob * OBW, (ob + 1) * OBW)
                            bps = prj_psum.tile([128, OBW], F32, tag="prj")
                            nc.tensor.matmul(
                                bps[:], ones_t[0:1, :], bias_t[0:1, obsl],
                                start=True, stop=True,
                            )
                            nc.vector.tensor_copy(bias_bc[:, obsl], bps[:])
                    if kb == 2 and pr + 1 < NPAIR:
                        tiles[pr + 1] = pair_dmas(pr + 1)
                    if kb == 4 and pr + 1 < NPAIR:
                        nc.sync.dma_start(
                            wt_ts[pr + 1][:], wt[:, (pr + 1) * H : (pr + 2) * H]
                        )
                    xt_t, xq_t, xa_t, _, _ = tiles[pr]
                    ksl = slice(kb * 128, (kb + 1) * 128)
                    s_ab = s_psum.tile([128, 2 * QB], F32, tag="s")
                    nc.tensor.matmul(
                        s_ab[:, 0:QB], xt_t[0:64, ksl], xq_t[0:64, :],
                        start=True, stop=True,
                    )
                    nc.tensor.matmul(
                        s_ab[:, QB : 2 * QB], xt_t[64:128, ksl], xq_t[64:128, :],
                        start=True, stop=True,
                    )
                    p_ab = p_pool.tile([128, 2 * QB], BF16, tag="p")
                    nc.scalar.activation(
                        p_ab[:], s_ab[:], mybir.ActivationFunctionType.Exp,
                        scale=SCALE,
                    )
                    cur = (gs, p_ab)
                if cur is not None:
                    pipe.append(cur)
                prev = pipe.pop(0) if (len(pipe) > SKEW or cur is None) and pipe else None
                if prev is not None:
                    gsp, pp = prev
                    prp, kbp = divmod(gsp, KBLKS)
                    if kbp == 0:
                        ctx_a = ctx_psum.tile([128, QSUB * AUG], F32, tag="ctx",
                                              name=f"ctxa{prp}")
                        ctx_b = ctx_psum.tile([128, QSUB * AUG], F32, tag="ctx",
                                              name=f"ctxb{prp}")
                        tiles[prp][3] = ctx_a
                        tiles[prp][4] = ctx_b
                    _, _, xa_t, ctx_a, ctx_b = tiles[prp]
                    for j, ctx_t in ((0, ctx_a), (1, ctx_b)):
                        asl = slice(j * KBLKS * AUG + kbp * AUG,
                                    j * KBLKS * AUG + (kbp + 1) * AUG)
                        for qs in range(QSUB):
                            # The 4 qs accumulation groups share one psum
                            # bank (= one 2KB zero region): only the tile's
                            # first matmul may set start (start marks the
                            # WHOLE region pending-zero, so later groups'
                            # first writes still overwrite-not-accumulate),
                            # and only the last sets stop.
                            nc.tensor.matmul(
                                ctx_t[:, qs * AUG : (qs + 1) * AUG],
                                pp[:, j * QB + qs * 128 : j * QB + (qs + 1) * 128],
                                xa_t[:, asl],
                                start=(kbp == 0 and qs == 0),
                                stop=(kbp == KBLKS - 1 and qs == QSUB - 1),
                            )
                    g = plan.get(gsp)
                    if g is not None:
                        with tc.tile_wait_until(floor_ms(gsp)):
                            emit_proj_group(g[0], g[1], g[2], prj_psum, "prj")
                    if kbp == KBLKS - 1:
                        emit_drain(prp, ctx_a, ctx_b)
    nc.compile()
    return nc


def _to_bf16(x):
    import ml_dtypes

    return np.asarray(x, np.float32).astype(ml_dtypes.bfloat16)


def shard_inputs(key, W_ctx, b_ctx):
    """Host-side prep of per-core input dicts (bf16 layouts)."""
    Bv = key.shape[0]
    cores_per_batch = NCORES // Bv

    key = np.asarray(key, dtype=np.float32)
    xh = key.reshape(Bv, L, NH, DH)
    # xt: [B, NPAIR, 128, L]; pair p rows 0:64 = head 2p, 64:128 = head 2p+1
    xt_full = np.ascontiguousarray(
        xh.transpose(0, 2, 3, 1).reshape(Bv, NPAIR, 2 * DH, L)
    )
    # xa: [B, NH, 128, KBLKS*AUG] with ones in column kb*AUG+DH, then pair-merged
    xa_full = np.empty((Bv, NH, 128, KBLKS * AUG), dtype=np.float32)
    xa_view = xa_full.reshape(Bv, NH, 128, KBLKS, AUG)
    xa_view[..., DH] = 1.0
    xa_view[..., 0:DH] = xh.reshape(Bv, KBLKS, 128, NH, DH).transpose(0, 3, 2, 1, 4)
    # [B, NPAIR, 2, 128, KA] -> [B, NPAIR, 128, 2*KA]
    ka = KBLKS * AUG
    xa_pair = np.ascontiguousarray(
        xa_full.reshape(Bv, NPAIR, 2, 128, ka).transpose(0, 1, 3, 2, 4)
        .reshape(Bv, NPAIR, 128, 2 * ka)
    )
    wt_host = np.ascontiguousarray(
        np.asarray(W_ctx, np.float32).T.reshape(HC, 128, H).transpose(1, 0, 2)
        .reshape(128, HC * H)
    )
    bias_host = np.asarray(b_ctx, np.float32).reshape(1, H)
    ones_host = np.ones((1, 128), dtype=np.float32)

    xt_b = _to_bf16(xt_full)
    xa_b = _to_bf16(xa_pair)
    wt_b = _to_bf16(wt_host)
    bias_b = _to_bf16(bias_host)
    ones_b = _to_bf16(ones_host)

    in_maps = []
    meta = []
    for c in range(NCORES):
        b = c // cores_per_batch
        qlo = (c % cores_per_batch) * QB
        in_maps.append(
            {
                "xt": xt_b[b],
                "xq": np.ascontiguousarray(xt_b[b][:, :, qlo : qlo + QB]),
                "xa": xa_b[b],
                "wt": wt_b,
                "bias": bias_b,
                "ones": ones_b,
            }
        )
        meta.append((b, qlo))
    return in_maps, meta


_NC_CACHE = {}


def kernel(key, W_ctx, b_ctx):
    from concourse.bass_utils import run_bass_kernel_spmd

    key = np.asarray(key, dtype=np.float32)
    if "nc" not in _NC_CACHE:
        _NC_CACHE["nc"] = build_nc()
    nc = _NC_CACHE["nc"]
    in_maps, meta = shard_inputs(key, W_ctx, b_ctx)
    res = run_bass_kernel_spmd(nc, in_maps, list(range(NCORES)))
    outf = np.empty((B, L, H), dtype=np.float32)
    for c, (b, qlo) in enumerate(meta):
        outf[b, qlo : qlo + QB] = res.results[c]["out"]
    return outf
